# revision 27
# baseline (speedup 1.0000x reference)
"""BiLSTM-CRF on 8 Trainium2 NeuronCores (Bass/Tile) — v2.

Cores 0/1: fwd/bwd LSTM with host-pretransposed embeddings (xeT), gate
columns host-permuted to [i f o | g] per H-half so sigmoid is one
instruction per half; emissions folded into the recurrence (em matmul
reuses the hT stationary, n=48). Emissions AllReduce over all 8 cores
(cores 2-7 contribute zeros).

Scan: all 8 cores. Pair p=c//2 owns batch slice [16p,16p+16); even core
runs the alpha (forward Viterbi) scan, odd core the gamma (backward)
scan. Layout: partitions = (ig*16+b) with i split 8 ways, free=[48 j,
6 il]; per step: TT add + reduce + 3-level partition max tree + score
redistribution (1 lane-aligned DVE copy + 7 small SBUF DMAs) + em add.
Scores to DRAM; pair AllGather; both cores compute tags for their 16 b
(argmax_j alpha+gamma-em via is_equal/min trick); tiny tags AllGather.
"""
import numpy as np

import concourse.bass as bass
import concourse.tile as tile
from concourse import mybir, bacc
from concourse.bass_utils import run_bass_kernel_spmd
from concourse.masks import make_identity

B, E, H, K, G = 64, 256, 512, 48, 2048
T = 512
V = 50000
N_CORES = 8
NTILE = T * B // 128  # 256 lstm tiles, 2 steps each
F32 = mybir.dt.float32
F32R = mybir.dt.float32r
I32 = mybir.dt.int32
AF = mybir.ActivationFunctionType
OP = mybir.AluOpType

# gate column permutation: new layout [i_h0 f_h0 o_h0 | g_h0 | i_h1 f_h1 o_h1 | g_h1]
# (orig rows: i=0:512, f=512:1024, g=1024:1536, o=1536:2048)
_GPERM = np.concatenate([
    np.arange(0, 256), np.arange(512, 768), np.arange(1536, 1792),
    np.arange(1024, 1280),
    np.arange(256, 512), np.arange(768, 1024), np.arange(1792, 2048),
    np.arange(1280, 1536)])


def _build_nc():
    nc = bacc.Bacc("TRN2", target_bir_lowering=False, debug=False,
                   num_devices=N_CORES)

    xeT_ap = nc.dram_tensor("xeT", [NTILE * 2 * 128, 128], F32,
                            kind="ExternalInput").ap()
    wih_ap = nc.dram_tensor("wih", [128, 2 * G], F32,
                            kind="ExternalInput").ap()
    whh_ap = nc.dram_tensor("whh", [128, 4 * G], F32,
                            kind="ExternalInput").ap()
    bias_ap = nc.dram_tensor("bias", [1, G], F32, kind="ExternalInput").ap()
    woutT_ap = nc.dram_tensor("woutT", [128, 4 * K], F32,
                              kind="ExternalInput").ap()
    boutr_ap = nc.dram_tensor("boutr", [128, K], F32,
                              kind="ExternalInput").ap()
    emstidx_ap = nc.dram_tensor("emstidx", [128, NTILE + 1], I32,
                                kind="ExternalInput").ap()
    transl_ap = nc.dram_tensor("transl", [128, K * 6], F32,
                               kind="ExternalInput").ap()
    pat8_ap = nc.dram_tensor("pat8", [128, 8 * 128], F32,
                             kind="ExternalInput").ap()
    sinit6_ap = nc.dram_tensor("sinit6", [128, 6], F32,
                               kind="ExternalInput").ap()
    scidx_ap = nc.dram_tensor("scidx", [128, T], I32,
                              kind="ExternalInput").ap()
    pridx_ap = nc.dram_tensor("pridx", [128, 2], I32,
                              kind="ExternalInput").ap()
    empidx_ap = nc.dram_tensor("empidx", [128, 4], I32,
                               kind="ExternalInput").ap()

    tags_ap = nc.dram_tensor("tags", [B, T], I32, kind="ExternalOutput").ap()

    EMR = T * B + 64  # + trash rows for the two half-junk em stores
    em_loc = nc.dram_tensor("em_loc", [EMR, K], F32)
    em_shared = nc.dram_tensor("em_shared", [EMR, K], F32,
                               addr_space="Shared")
    score_loc = nc.dram_tensor("score_loc", [T * 16, K], F32)
    score_gath = nc.dram_tensor("score_gath", [N_CORES * T * 16, K], F32,
                                addr_space="Shared")
    score_pair = nc.dram_tensor("score_pair", [2 * T * 16, K], F32)
    em_pair = nc.dram_tensor("em_pair", [T * 16, K], F32)
    tags_loc = nc.dram_tensor("tags_loc", [16, T], I32)
    tags_gath = nc.dram_tensor("tags_gath", [N_CORES * 16, T], I32,
                               addr_space="Shared")

    g_all = [list(range(N_CORES))]
    g_pair = [[2 * p, 2 * p + 1] for p in range(4)]

    with tile.TileContext(nc) as tc:
        with tc.tile_pool(name="const", bufs=1) as cp:
            ident = cp.tile([128, 128], F32)
            make_identity(nc, ident[:])

            stage_ctx = tc.tile_pool(name="stage", bufs=1)
            sp0 = stage_ctx.__enter__()
            wih_f = sp0.tile([128, 2, G], F32)
            nc.sync.dma_start(wih_f[:], wih_ap[:, :])
            wih_r = cp.tile([128, 2, G], F32R)
            nc.vector.tensor_copy(wih_r[:], wih_f[:])
            whh_f = sp0.tile([128, 4, G], F32)
            nc.sync.dma_start(whh_f[:], whh_ap[:, :])
            whh_r = cp.tile([128, 4, G], F32R)
            nc.vector.tensor_copy(whh_r[:], whh_f[:])
            bias_f = sp0.tile([1, G], F32)
            nc.sync.dma_start(bias_f[:], bias_ap[:, :])
            bias_r = cp.tile([1, G], F32R)
            nc.vector.tensor_copy(bias_r[:], bias_f[:])
            woutT_f = sp0.tile([128, 4, K], F32)
            nc.sync.dma_start(woutT_f[:], woutT_ap[:, :])
            woutT_r = cp.tile([128, 4, K], F32R)
            nc.vector.tensor_copy(woutT_r[:], woutT_f[:])
            stage_ctx.__exit__(None, None, None)
            boutr_sb = cp.tile([128, K], F32)
            nc.sync.dma_start(boutr_sb[:], boutr_ap[:, :])
            emstidx_sb = cp.tile([128, NTILE + 1], I32)
            nc.sync.dma_start(emstidx_sb[:], emstidx_ap[:, :])
            transl_sb = cp.tile([128, 6, K], F32)
            nc.scalar.dma_start(transl_sb[:], transl_ap[:, :])
            pat8_sb = cp.tile([128, 8, 128], F32)
            nc.scalar.dma_start(pat8_sb[:], pat8_ap[:, :])
            sinit6_sb = cp.tile([128, 6], F32)
            nc.scalar.dma_start(sinit6_sb[:], sinit6_ap[:, :])
            scidx_sb = cp.tile([128, T], I32)
            nc.scalar.dma_start(scidx_sb[:], scidx_ap[:, :])
            pridx_sb = cp.tile([128, 2], I32)
            nc.scalar.dma_start(pridx_sb[:], pridx_ap[:, :])
            empidx_sb = cp.tile([128, 4], I32)
            nc.scalar.dma_start(empidx_sb[:], empidx_ap[:, :])

            ones_r = cp.tile([1, 128], F32R)
            nc.vector.memset(ones_r[:].bitcast(F32), 1.0)

            jshift = cp.tile([128, K], F32)
            jshift_i = cp.tile([128, K], I32)
            nc.gpsimd.iota(jshift_i[:], pattern=[[1, K]], base=0,
                           channel_multiplier=0)
            nc.vector.tensor_copy(jshift[:], jshift_i[:])
            nc.vector.tensor_scalar_sub(jshift[:], jshift[:], 1000.0)

            em_view = bass.AP(tensor=em_loc.ap().tensor, offset=0,
                              ap=[[K, EMR], [1, K]])

            # ---------------- LSTM + emissions ----------------
            with tc.tile_pool(name="lstm", bufs=2) as lp, \
                 tc.tile_pool(name="psg", bufs=3, space="PSUM") as psg, \
                 tc.tile_pool(name="psa", bufs=1, space="PSUM") as psa:

                tr_ps = psa.tile([128, 256], F32, tag="tr", bufs=1,
                                 name="trP")
                emt = psa.tile([128, 128], F32, tag="emt", bufs=1,
                               name="emtP")
                hT_a = lp.tile([128, 4, 128], F32R, tag="hT", bufs=2,
                               name="hTinitA")
                nc.vector.memset(hT_a[:].bitcast(F32), 0.0)
                hT = lp.tile([128, 4, 128], F32R, tag="hT", bufs=2,
                             name="hT0")
                nc.vector.memset(hT[:].bitcast(F32), 0.0)
                cst = lp.tile([B, H], F32, tag="cst", bufs=2, name="cst0")
                nc.vector.memset(cst[:], 0.0)

                for kt in range(NTILE):
                    xet_f = lp.tile([128, 2, 128], F32, tag="xetf", bufs=3,
                                    name=f"xetf{kt}")
                    nc.sync.dma_start(xet_f[:], bass.AP(
                        tensor=xeT_ap.tensor, offset=kt * 2 * 128 * 128,
                        ap=[[128, 128], [128 * 128, 2], [1, 128]]))
                    xet = lp.tile([128, 2, 128], F32R, tag="xet", bufs=3,
                                  name=f"xet{kt}")
                    nc.vector.tensor_copy(xet[:], xet_f[:])

                    gxh = [psg.tile([128, 1024], F32, tag="gxh",
                                    name=f"gx{kt}_{h}") for h in range(2)]

                    # bias + input gates
                    for q in range(4):
                        dst = gxh[q // 2][:, (q % 2) * 512:(q % 2) * 512 + 512]
                        nc.tensor.matmul(dst, ones_r[0:1, :],
                                         bias_r[0:1, q * 512:(q + 1) * 512],
                                         start=True, stop=False,
                                         skip_group_check=True)
                    for e in range(2):
                        for q in range(4):
                            dst = gxh[q // 2][:,
                                              (q % 2) * 512:(q % 2) * 512 + 512]
                            nc.tensor.matmul(
                                dst, xet[:, e, :],
                                wih_r[:, e, q * 512:(q + 1) * 512],
                                start=False, stop=False,
                                skip_group_check=True)

                    for half in range(2):
                        s = 2 * kt + half
                        ro = 64 * half
                        nhalf = (s + 1) % 2
                        for kk in range(4):
                            for q in range(4):
                                dst = gxh[q // 2][
                                    :, (q % 2) * 512:(q % 2) * 512 + 512]
                                nc.tensor.matmul(
                                    dst, hT[:, kk, :],
                                    whh_r[:, kk, q * 512:(q + 1) * 512],
                                    start=False, stop=(kk == 3),
                                    skip_group_check=True)
                            nc.tensor.matmul(
                                emt[:, 64 * half:64 * half + K],
                                hT[:, kk, :], woutT_r[:, kk, :],
                                start=(kk == 0), stop=(kk == 3),
                                skip_group_check=True)

                        cst_new = lp.tile([B, H], F32, tag="cst", bufs=2,
                                          name=f"cst{s + 1}")
                        hT_new = lp.tile([128, 4, 128], F32R, tag="hT",
                                         bufs=2, name=f"hT{s + 1}")
                        for hf in range(2):
                            gsrc = gxh[hf]
                            hs = slice(256 * hf, 256 * hf + 256)
                            sio = lp.tile([B, 768], F32, tag=f"sio{hf}",
                                          bufs=2, name=f"sio{s}_{hf}")
                            nc.scalar.activation(sio[:],
                                                 gsrc[ro:ro + 64, 0:768],
                                                 AF.Sigmoid)
                            tg = lp.tile([B, 256], F32, tag=f"tg{hf}",
                                         bufs=2, name=f"tg{s}_{hf}")
                            nc.scalar.activation(tg[:],
                                                 gsrc[ro:ro + 64, 768:1024],
                                                 AF.Tanh)
                            ig = lp.tile([B, 256], F32, tag=f"ig{hf}",
                                         bufs=2, name=f"ig{s}_{hf}")
                            nc.vector.tensor_mul(ig[:], sio[:, 0:256], tg[:])
                            fc = lp.tile([B, 256], F32, tag=f"fc{hf}",
                                         bufs=2, name=f"fc{s}_{hf}")
                            nc.gpsimd.tensor_mul(fc[:], sio[:, 256:512],
                                                 cst[:, hs])
                            nc.gpsimd.tensor_add(cst_new[:, hs], ig[:],
                                                 fc[:])
                            tcc = lp.tile([B, 256], F32, tag=f"tcc{hf}",
                                          bufs=2, name=f"tcc{s}_{hf}")
                            nc.scalar.activation(tcc[:], cst_new[:, hs],
                                                 AF.Tanh)
                            hh = lp.tile([B, 256], F32, tag=f"hh{hf}",
                                         bufs=2, name=f"hh{s}_{hf}")
                            nc.vector.tensor_mul(hh[:], sio[:, 512:768],
                                                 tcc[:])
                            for c2 in range(2):
                                nc.tensor.transpose(
                                    tr_ps[:, (2 * hf + c2) * 64:
                                          (2 * hf + c2 + 1) * 64],
                                    hh[:, c2 * 128:(c2 + 1) * 128],
                                    ident[0:64, 0:64])
                            dst_hf = bass.AP(
                                tensor=hT_new[:].tensor,
                                offset=hT_new[:].offset + nhalf * 64
                                + 2 * hf * 128,
                                ap=[hT_new[:].ap[0], [128, 2], [1, 64]])
                            nc.vector.tensor_copy(
                                dst_hf,
                                tr_ps[:, 2 * hf * 64:
                                      (2 * hf + 2) * 64].rearrange(
                                    "p (a b) -> p a b", a=2))
                        cst = cst_new
                        hT = hT_new

                    em_sb = lp.tile([128, K], F32, tag="emsb", bufs=2,
                                    name=f"emsb{kt}")
                    nc.vector.tensor_tensor(em_sb[0:64, :],
                                            emt[0:64, 0:K],
                                            boutr_sb[0:64, :], op=OP.add)
                    nc.vector.tensor_tensor(em_sb[64:128, :],
                                            emt[64:128, 64:64 + K],
                                            boutr_sb[64:128, :], op=OP.add)
                    nc.gpsimd.indirect_dma_start(
                        out=em_view, out_offset=bass.IndirectOffsetOnAxis(
                            ap=emstidx_sb[:, kt:kt + 1], axis=0),
                        in_=em_sb[:], in_offset=None)

                # final emission for step T-1 (h from last half, m-half 0)
                for kk in range(4):
                    nc.tensor.matmul(emt[:, 0:K], hT[:, kk, :],
                                     woutT_r[:, kk, :], start=(kk == 0),
                                     stop=(kk == 3), skip_group_check=True)
                em_f = lp.tile([128, K], F32, tag="emsb", bufs=2,
                               name="emsbF")
                nc.vector.tensor_tensor(em_f[:], emt[:, 0:K],
                                        boutr_sb[:], op=OP.add)
                nc.gpsimd.indirect_dma_start(
                    out=em_view, out_offset=bass.IndirectOffsetOnAxis(
                        ap=emstidx_sb[:, NTILE:NTILE + 1], axis=0),
                    in_=em_f[:], in_offset=None)

            nc.gpsimd.collective_compute(
                "AllReduce", OP.add, replica_groups=g_all,
                ins=[em_loc.ap().opt()], outs=[em_shared.ap().opt()])

            # ---------------- max-plus scans (all cores) ----------------
            em6_view = bass.AP(tensor=em_shared.ap().tensor, offset=0,
                               ap=[[6, EMR * 8], [1, 6]])
            with tc.tile_pool(name="scan", bufs=2) as sp, \
                 tc.tile_pool(name="psc", bufs=2, space="PSUM") as psc:
                embk = {}

                def gather_step(k):
                    t_ = sp.tile([128, 6], F32, tag="embk", bufs=8,
                                 name=f"embk{k}")
                    nc.gpsimd.indirect_dma_start(
                        out=t_[:], out_offset=None, in_=em6_view,
                        in_offset=bass.IndirectOffsetOnAxis(
                            ap=scidx_sb[:, k:k + 1], axis=0))
                    embk[k] = t_

                def replicate(s6, k):
                    # score_rep[p=(jg',b), jg*6+jl] = s6[(jg,b), jl] via 8
                    # tiny MMs against the b-selector stationary (PE idle
                    # during the scan otherwise)
                    rep = psc.tile([128, K], F32, tag="srep",
                                   name=f"srep{k}")
                    for jg in range(8):
                        nc.tensor.matmul(rep[:, 6 * jg:6 * jg + 6],
                                         pat8_sb[:, jg, :], s6[:],
                                         start=True, stop=True,
                                         skip_group_check=True)
                    return rep

                for kk_ in range(6):
                    gather_step(kk_)
                s6 = sp.tile([128, 6], F32, tag="s6", bufs=2, name="s6_0")
                nc.vector.tensor_tensor(s6[:], sinit6_sb[:],
                                        embk[0][:], op=OP.add)
                nc.scalar.dma_start(
                    bass.AP(tensor=score_loc.ap().tensor, offset=0,
                            ap=[[6, 8], [K, 16], [1, 6]]), s6[:])
                srep = replicate(s6, 0)

                for k in range(1, T):
                    if k + 5 < T:
                        gather_step(k + 5)
                    cand = sp.tile([128, 6, K], F32, tag="cand", bufs=2,
                                   name=f"cand{k}")
                    sr_b = bass.AP(tensor=srep[:].tensor,
                                   offset=srep[:].offset,
                                   ap=[srep[:].ap[0], [0, 6], [1, K]])
                    nc.vector.tensor_tensor(cand[:], transl_sb[:], sr_b,
                                            op=OP.add)
                    red = sp.tile([128, 6], F32, tag="red", bufs=2,
                                  name=f"red{k}")
                    nc.vector.tensor_reduce(red[:], cand[:],
                                            axis=mybir.AxisListType.X,
                                            op=OP.max)
                    s6 = sp.tile([128, 6], F32, tag="s6", bufs=2,
                                 name=f"s6_{k}")
                    nc.vector.tensor_tensor(s6[:], red[:],
                                            embk.pop(k)[:], op=OP.add)
                    nc.scalar.dma_start(
                        bass.AP(tensor=score_loc.ap().tensor,
                                offset=k * 16 * K,
                                ap=[[6, 8], [K, 16], [1, 6]]), s6[:])
                    if k < T - 1:
                        srep = replicate(s6, k)

            nc.gpsimd.collective_compute(
                "AllGather", OP.bypass, replica_groups=g_all,
                ins=[score_loc.ap().opt()], outs=[score_gath.ap().opt()])

            # ---------------- tags (argmax alpha+gamma-em) ----------------
            with tc.tile_pool(name="fin", bufs=2) as fp:
                # bulk-copy this pair's alpha/gamma rank blocks + em slice
                # to local DRAM (per-core variance only in gather offsets)
                for r_ in range(2):
                    g_ = fp.tile([128, 3072], F32, tag="blk", bufs=2,
                                 name=f"blk{r_}")
                    nc.gpsimd.indirect_dma_start(
                        out=g_[:], out_offset=None,
                        in_=bass.AP(tensor=score_gath.ap().tensor, offset=0,
                                    ap=[[3072, N_CORES * T * 16 * K // 3072],
                                        [1, 3072]]),
                        in_offset=bass.IndirectOffsetOnAxis(
                            ap=pridx_sb[:, r_:r_ + 1], axis=0))
                    nc.sync.dma_start(
                        bass.AP(tensor=score_pair.ap().tensor,
                                offset=r_ * T * 16 * K,
                                ap=[[3072, 128], [1, 3072]]), g_[:])
                for tc_ in range(4):
                    ge_ = fp.tile([128, 768], F32, tag="eblk", bufs=2,
                                  name=f"eblk{tc_}")
                    nc.gpsimd.indirect_dma_start(
                        out=ge_[:], out_offset=None,
                        in_=bass.AP(tensor=em_shared.ap().tensor, offset=0,
                                    ap=[[768, T * B * K // 768], [1, 768]]),
                        in_offset=bass.IndirectOffsetOnAxis(
                            ap=empidx_sb[:, tc_:tc_ + 1], axis=0))
                    nc.sync.dma_start(
                        bass.AP(tensor=em_pair.ap().tensor,
                                offset=tc_ * 128 * 768,
                                ap=[[768, 128], [1, 768]]), ge_[:])

                tags_i = fp.tile([128, 64], I32, tag="tagsi", bufs=1,
                                 name="tagsi")
                for b2 in range(2):
                    al = fp.tile([128, 32, K], F32, tag="al", bufs=2,
                                 name=f"al{b2}")
                    ga = fp.tile([128, 32, K], F32, tag="ga", bufs=2,
                                 name=f"ga{b2}")
                    em2 = fp.tile([128, 32, K], F32, tag="em2", bufs=2,
                                  name=f"em2_{b2}")
                    for tg in range(8):
                        t0 = tg * 64 + b2 * 32
                        k_lo = 511 - t0 - 31
                        nc.sync.dma_start(
                            al[16 * tg:16 * tg + 16, :, :], bass.AP(
                                tensor=score_pair.ap().tensor,
                                offset=t0 * 16 * K,
                                ap=[[K, 16], [16 * K, 32], [1, K]]))
                        nc.scalar.dma_start(
                            ga[16 * tg:16 * tg + 16, :, :], bass.AP(
                                tensor=score_pair.ap().tensor,
                                offset=(T + k_lo) * 16 * K,
                                ap=[[K, 16], [16 * K, 32], [1, K]]))
                        nc.gpsimd.dma_start(
                            em2[16 * tg:16 * tg + 16, :, :], bass.AP(
                                tensor=em_pair.ap().tensor,
                                offset=t0 * 16 * K,
                                ap=[[K, 16], [16 * K, 32], [1, K]]))
                    tot = fp.tile([128, 32, K], F32, tag="tot", bufs=2,
                                  name=f"tot{b2}")
                    ga_rev = bass.AP(
                        tensor=ga[:].tensor, offset=ga[:].offset + 31 * K,
                        ap=[ga[:].ap[0], [-K, 32], [1, K]])
                    nc.vector.tensor_tensor(tot[:], al[:], ga_rev, op=OP.add)
                    nc.vector.tensor_tensor(tot[:], tot[:], em2[:],
                                            op=OP.subtract)
                    mx = fp.tile([128, 32], F32, tag="mx", bufs=2,
                                 name=f"mx{b2}")
                    nc.vector.tensor_reduce(mx[:], tot[:],
                                            axis=mybir.AxisListType.X,
                                            op=OP.max)
                    msk = fp.tile([128, 32, K], F32, tag="msk", bufs=2,
                                  name=f"msk{b2}")
                    nc.vector.tensor_tensor(
                        msk[:], tot[:],
                        bass.AP(tensor=mx[:].tensor, offset=mx[:].offset,
                                ap=[mx[:].ap[0], [1, 32], [0, K]]),
                        op=OP.is_equal)
                    nc.vector.tensor_tensor(
                        msk[:], msk[:],
                        bass.AP(tensor=jshift[:].tensor,
                                offset=jshift[:].offset,
                                ap=[jshift[:].ap[0], [0, 32], [1, K]]),
                        op=OP.mult)
                    jm = fp.tile([128, 32], F32, tag="jm", bufs=2,
                                 name=f"jm{b2}")
                    nc.vector.tensor_reduce(jm[:], msk[:],
                                            axis=mybir.AxisListType.X,
                                            op=OP.min)
                    nc.vector.tensor_scalar_add(
                        tags_i[:, b2 * 32:b2 * 32 + 32], jm[:], 1000.0)

                # tags_i [p=(tg,b), u=b2*32+tl] -> tags_loc[b, tg*64+u]
                nc.sync.dma_start(
                    bass.AP(tensor=tags_loc.ap().tensor, offset=0,
                            ap=[[64, 8], [T, 16], [1, 64]]), tags_i[:])

            nc.gpsimd.collective_compute(
                "AllGather", OP.bypass, replica_groups=g_all,
                ins=[tags_loc.ap().opt()], outs=[tags_gath.ap().opt()])

            with tc.tile_pool(name="out", bufs=1) as op_:
                tags_sb = op_.tile([B, T], I32)
                for p4 in range(4):
                    nc.sync.dma_start(
                        tags_sb[16 * p4:16 * p4 + 16, :],
                        bass.AP(tensor=tags_gath.ap().tensor,
                                offset=(2 * p4) * 16 * T,
                                ap=[[T, 16], [1, T]]))
                nc.sync.dma_start(tags_ap[:, :], tags_sb[:])

    nc.compile()
    return nc


def _host_prep(inputs):
    x = np.asarray(inputs["x"]).astype(np.int64)
    emb = np.asarray(inputs["emb"], np.float32)
    trans = np.asarray(inputs["crf_trans"], np.float32)

    maps = []
    p = np.arange(128)
    for core in range(N_CORES):
        m = {}
        is_lstm = core < 2
        d = "f" if core == 0 else "b"
        if is_lstm:
            x_eff = x if core == 0 else np.ascontiguousarray(x[:, ::-1])
            xe = emb[x_eff]                       # [B, T, E]
            xseq = xe.transpose(1, 0, 2).reshape(NTILE, 128, E)
            xeT = np.ascontiguousarray(
                xseq.transpose(0, 2, 1)).reshape(NTILE * 2 * 128, 128)
            m["xeT"] = xeT
            wih = np.asarray(inputs[f"w_ih_{d}"], np.float32)[_GPERM].T
            m["wih"] = np.ascontiguousarray(
                wih.reshape(2, 128, G).transpose(1, 0, 2)).reshape(128, 2 * G)
            whh = np.asarray(inputs[f"w_hh_{d}"], np.float32)[_GPERM].T
            m["whh"] = np.ascontiguousarray(
                whh.reshape(4, 128, G).transpose(1, 0, 2)).reshape(128, 4 * G)
            m["bias"] = np.asarray(
                inputs[f"b_{d}"], np.float32)[_GPERM].reshape(1, G)
            wo = np.asarray(inputs["w_out"], np.float32)
            half = wo[:, :H] if core == 0 else wo[:, H:]
            m["woutT"] = np.ascontiguousarray(
                half.T.reshape(4, 128, K).transpose(1, 0, 2)).reshape(
                128, 4 * K)
            m["boutr"] = (np.tile(np.asarray(inputs["b_out"], np.float32),
                                  (128, 1)) if core == 0
                          else np.zeros((128, K), np.float32))
        else:
            m["xeT"] = np.zeros((NTILE * 2 * 128, 128), np.float32)
            m["wih"] = np.zeros((128, 2 * G), np.float32)
            m["whh"] = np.zeros((128, 4 * G), np.float32)
            m["bias"] = np.zeros((1, G), np.float32)
            m["woutT"] = np.zeros((128, 4 * K), np.float32)
            m["boutr"] = np.zeros((128, K), np.float32)

        # em store indices: tile kt rows r: step = 2kt-1 + r//64 (fwd t=step,
        # bwd t = T-1-step), row = t*B + b, b = r % 64
        tmap = (np.arange(T) if core % 2 == 0 else T - 1 - np.arange(T))
        st = np.empty((128, NTILE + 1), np.int64)
        lstm_tmap = np.arange(T) if core != 1 else T - 1 - np.arange(T)
        trash = T * B + (p % 64)
        for kt in range(NTILE):
            step = 2 * kt - 1 + p // 64
            st[:, kt] = np.where(step < 0, trash,
                                 lstm_tmap[np.clip(step, 0, T - 1)] * B
                                 + (p % 64))
        st[:, NTILE] = np.where(p < 64, lstm_tmap[T - 1] * B + (p % 64),
                                trash)
        m["emstidx"] = st.astype(np.int32)

        # scan inputs: pair owns b in [16*pair, 16*pair+16)
        pair = core // 2
        is_alpha = (core % 2 == 0)
        ig = p // 16
        bl = p % 16
        bg = 16 * pair + bl
        tr = trans if is_alpha else np.ascontiguousarray(trans.T)
        # transl[p=(jg,b), il, i] = tr[i, j=jg*6+il]  (j-split layout)
        m["transl"] = np.ascontiguousarray(
            tr.T[(ig[:, None] * 6 + np.arange(6)[None, :])]).reshape(
            128, 6 * K)
        c_ = np.arange(128)[:, None, None]
        jg_ = np.arange(8)[None, :, None]
        m_ = np.arange(128)[None, None, :]
        m["pat8"] = (c_ == jg_ * 16 + m_ % 16).astype(
            np.float32).reshape(128, 8 * 128)
        sv = np.asarray(inputs["crf_start" if is_alpha else "crf_end"],
                        np.float32)
        s6 = np.empty((128, 6), np.float32)
        for il in range(6):
            s6[:, il] = sv[ig * 6 + il]
        m["sinit6"] = s6
        # scidx[p, k] = row of em viewed [T*B*8, 6]: t_order(k)*B*8 + bg*8+ig
        korder = np.arange(T) if is_alpha else T - 1 - np.arange(T)
        m["scidx"] = (korder[None, :] * (B * 8) + (bg * 8 + ig)[:, None]
                      ).astype(np.int32)
        # tags bulk-copy offsets: pridx (rank blocks, 3072-el units),
        # empidx (em t-rows for pair's b-slice, 768-el units)
        pr = np.empty((128, 2), np.int64)
        pr[:, 0] = 2 * pair * 128 + p
        pr[:, 1] = (2 * pair + 1) * 128 + p
        m["pridx"] = pr.astype(np.int32)
        emp = np.empty((128, 4), np.int64)
        for tc_ in range(4):
            emp[:, tc_] = 4 * (tc_ * 128 + p) + pair
        m["empidx"] = emp.astype(np.int32)
        maps.append(m)
    return maps


_NC_CACHE = {}


def _get_nc():
    if "nc" not in _NC_CACHE:
        _NC_CACHE["nc"] = _build_nc()
    return _NC_CACHE["nc"]


def kernel(**inputs):
    nc = _get_nc()
    maps = _host_prep(inputs)
    res = run_bass_kernel_spmd(nc, maps, core_ids=list(range(N_CORES)))
    return res.results[0]["tags"].astype(np.int32)


# revision 28
# speedup vs baseline: 1.8215x; 1.8215x over previous
"""BiLSTM-CRF on 8 Trainium2 NeuronCores (Bass/Tile) — v2.

Cores 0/1: fwd/bwd LSTM with host-pretransposed embeddings (xeT), gate
columns host-permuted to [i f o | g] per H-half so sigmoid is one
instruction per half; emissions folded into the recurrence (em matmul
reuses the hT stationary, n=48). Emissions AllReduce over all 8 cores
(cores 2-7 contribute zeros).

Scan: all 8 cores. Pair p=c//2 owns batch slice [16p,16p+16); even core
runs the alpha (forward Viterbi) scan, odd core the gamma (backward)
scan. Layout: partitions = (ig*16+b) with i split 8 ways, free=[48 j,
6 il]; per step: TT add + reduce + 3-level partition max tree + score
redistribution (1 lane-aligned DVE copy + 7 small SBUF DMAs) + em add.
Scores to DRAM; pair AllGather; both cores compute tags for their 16 b
(argmax_j alpha+gamma-em via is_equal/min trick); tiny tags AllGather.
"""
import numpy as np

import concourse.bass as bass
import concourse.tile as tile
from concourse import mybir, bacc
from concourse.bass_utils import run_bass_kernel_spmd
from concourse.masks import make_identity

B, E, H, K, G = 64, 256, 512, 48, 2048
T = 512
V = 50000
N_CORES = 8
NTILE = T * B // 128  # 256 lstm tiles, 2 steps each
F32 = mybir.dt.float32
F32R = mybir.dt.float32r
I32 = mybir.dt.int32
AF = mybir.ActivationFunctionType
OP = mybir.AluOpType

# gate column permutation: new layout [i_h0 f_h0 o_h0 | g_h0 | i_h1 f_h1 o_h1 | g_h1]
# (orig rows: i=0:512, f=512:1024, g=1024:1536, o=1536:2048)
_GPERM = np.concatenate([
    np.arange(0, 256), np.arange(512, 768), np.arange(1536, 1792),
    np.arange(1024, 1280),
    np.arange(256, 512), np.arange(768, 1024), np.arange(1792, 2048),
    np.arange(1280, 1536)])


def _build_nc():
    nc = bacc.Bacc("TRN2", target_bir_lowering=False, debug=False,
                   num_devices=N_CORES)

    xeT_ap = nc.dram_tensor("xeT", [NTILE * 2 * 128, 128], F32,
                            kind="ExternalInput").ap()
    wih_ap = nc.dram_tensor("wih", [128, 2 * G], F32,
                            kind="ExternalInput").ap()
    whh_ap = nc.dram_tensor("whh", [128, 4 * G], F32,
                            kind="ExternalInput").ap()
    bias_ap = nc.dram_tensor("bias", [1, G], F32, kind="ExternalInput").ap()
    woutT_ap = nc.dram_tensor("woutT", [128, 4 * K], F32,
                              kind="ExternalInput").ap()
    boutr_ap = nc.dram_tensor("boutr", [128, K], F32,
                              kind="ExternalInput").ap()
    emstidx_ap = nc.dram_tensor("emstidx", [128, NTILE + 1], I32,
                                kind="ExternalInput").ap()
    transl_ap = nc.dram_tensor("transl", [128, K * 6], F32,
                               kind="ExternalInput").ap()
    patb_ap = nc.dram_tensor("patb", [128, 128], F32,
                             kind="ExternalInput").ap()
    bmask_ap = nc.dram_tensor("bmask", [128, K], F32,
                              kind="ExternalInput").ap()
    sinit6_ap = nc.dram_tensor("sinit6", [128, 6], F32,
                               kind="ExternalInput").ap()
    scidx_ap = nc.dram_tensor("scidx", [128, T], I32,
                              kind="ExternalInput").ap()
    pridx_ap = nc.dram_tensor("pridx", [128, 2], I32,
                              kind="ExternalInput").ap()
    empidx_ap = nc.dram_tensor("empidx", [128, 4], I32,
                               kind="ExternalInput").ap()

    tags_ap = nc.dram_tensor("tags", [B, T], I32, kind="ExternalOutput").ap()

    EMR = T * B + 64  # + trash rows for the two half-junk em stores
    em_loc = nc.dram_tensor("em_loc", [EMR, K], F32)
    em_shared = nc.dram_tensor("em_shared", [EMR, K], F32,
                               addr_space="Shared")
    score_loc = nc.dram_tensor("score_loc", [T * 16, K], F32)
    score_gath = nc.dram_tensor("score_gath", [N_CORES * T * 16, K], F32,
                                addr_space="Shared")
    score_pair = nc.dram_tensor("score_pair", [2 * T * 16, K], F32)
    em_pair = nc.dram_tensor("em_pair", [T * 16, K], F32)
    tags_loc = nc.dram_tensor("tags_loc", [16, T], I32)
    tags_gath = nc.dram_tensor("tags_gath", [N_CORES * 16, T], I32,
                               addr_space="Shared")

    g_all = [list(range(N_CORES))]
    g_pair = [[2 * p, 2 * p + 1] for p in range(4)]

    with tile.TileContext(nc) as tc:
        with tc.tile_pool(name="const", bufs=1) as cp:
            ident = cp.tile([128, 128], F32)
            make_identity(nc, ident[:])

            stage_ctx = tc.tile_pool(name="stage", bufs=1)
            sp0 = stage_ctx.__enter__()
            wih_f = sp0.tile([128, 2, G], F32)
            nc.sync.dma_start(wih_f[:], wih_ap[:, :])
            wih_r = cp.tile([128, 2, G], F32R)
            nc.vector.tensor_copy(wih_r[:], wih_f[:])
            whh_f = sp0.tile([128, 4, G], F32)
            nc.sync.dma_start(whh_f[:], whh_ap[:, :])
            whh_r = cp.tile([128, 4, G], F32R)
            nc.vector.tensor_copy(whh_r[:], whh_f[:])
            bias_f = sp0.tile([1, G], F32)
            nc.sync.dma_start(bias_f[:], bias_ap[:, :])
            bias_r = cp.tile([1, G], F32R)
            nc.vector.tensor_copy(bias_r[:], bias_f[:])
            woutT_f = sp0.tile([128, 4, K], F32)
            nc.sync.dma_start(woutT_f[:], woutT_ap[:, :])
            woutT_r = cp.tile([128, 4, K], F32R)
            nc.vector.tensor_copy(woutT_r[:], woutT_f[:])
            stage_ctx.__exit__(None, None, None)
            boutr_sb = cp.tile([128, K], F32)
            nc.sync.dma_start(boutr_sb[:], boutr_ap[:, :])
            emstidx_sb = cp.tile([128, NTILE + 1], I32)
            nc.sync.dma_start(emstidx_sb[:], emstidx_ap[:, :])
            transl_sb = cp.tile([128, 6, K], F32)
            nc.scalar.dma_start(transl_sb[:], transl_ap[:, :])
            patb_sb = cp.tile([128, 128], F32)
            nc.scalar.dma_start(patb_sb[:], patb_ap[:, :])
            bmask_sb = cp.tile([128, K], F32)
            nc.scalar.dma_start(bmask_sb[:], bmask_ap[:, :])
            sinit6_sb = cp.tile([128, 6], F32)
            nc.scalar.dma_start(sinit6_sb[:], sinit6_ap[:, :])
            scidx_sb = cp.tile([128, T], I32)
            nc.scalar.dma_start(scidx_sb[:], scidx_ap[:, :])
            pridx_sb = cp.tile([128, 2], I32)
            nc.scalar.dma_start(pridx_sb[:], pridx_ap[:, :])
            empidx_sb = cp.tile([128, 4], I32)
            nc.scalar.dma_start(empidx_sb[:], empidx_ap[:, :])

            ones_r = cp.tile([1, 128], F32R)
            nc.vector.memset(ones_r[:].bitcast(F32), 1.0)

            jshift = cp.tile([128, K], F32)
            jshift_i = cp.tile([128, K], I32)
            nc.gpsimd.iota(jshift_i[:], pattern=[[1, K]], base=0,
                           channel_multiplier=0)
            nc.vector.tensor_copy(jshift[:], jshift_i[:])
            nc.vector.tensor_scalar_sub(jshift[:], jshift[:], 1000.0)

            em_view = bass.AP(tensor=em_loc.ap().tensor, offset=0,
                              ap=[[K, EMR], [1, K]])

            # ---------------- LSTM + emissions ----------------
            with tc.tile_pool(name="lstm", bufs=2) as lp, \
                 tc.tile_pool(name="psg", bufs=3, space="PSUM") as psg, \
                 tc.tile_pool(name="psa", bufs=1, space="PSUM") as psa:

                tr_ps = psa.tile([128, 256], F32, tag="tr", bufs=1,
                                 name="trP")
                emt = psa.tile([128, 128], F32, tag="emt", bufs=1,
                               name="emtP")
                hT_a = lp.tile([128, 4, 128], F32R, tag="hT", bufs=2,
                               name="hTinitA")
                nc.vector.memset(hT_a[:].bitcast(F32), 0.0)
                hT = lp.tile([128, 4, 128], F32R, tag="hT", bufs=2,
                             name="hT0")
                nc.vector.memset(hT[:].bitcast(F32), 0.0)
                cst = lp.tile([B, H], F32, tag="cst", bufs=2, name="cst0")
                nc.vector.memset(cst[:], 0.0)

                for kt in range(NTILE):
                    xet_f = lp.tile([128, 2, 128], F32, tag="xetf", bufs=3,
                                    name=f"xetf{kt}")
                    nc.sync.dma_start(xet_f[:], bass.AP(
                        tensor=xeT_ap.tensor, offset=kt * 2 * 128 * 128,
                        ap=[[128, 128], [128 * 128, 2], [1, 128]]))
                    xet = lp.tile([128, 2, 128], F32R, tag="xet", bufs=3,
                                  name=f"xet{kt}")
                    nc.vector.tensor_copy(xet[:], xet_f[:])

                    gxh = [psg.tile([128, 1024], F32, tag="gxh",
                                    name=f"gx{kt}_{h}") for h in range(2)]

                    # bias + input gates
                    for q in range(4):
                        dst = gxh[q // 2][:, (q % 2) * 512:(q % 2) * 512 + 512]
                        nc.tensor.matmul(dst, ones_r[0:1, :],
                                         bias_r[0:1, q * 512:(q + 1) * 512],
                                         start=True, stop=False,
                                         skip_group_check=True)
                    for e in range(2):
                        for q in range(4):
                            dst = gxh[q // 2][:,
                                              (q % 2) * 512:(q % 2) * 512 + 512]
                            nc.tensor.matmul(
                                dst, xet[:, e, :],
                                wih_r[:, e, q * 512:(q + 1) * 512],
                                start=False, stop=False,
                                skip_group_check=True)

                    for half in range(2):
                        s = 2 * kt + half
                        ro = 64 * half
                        nhalf = (s + 1) % 2
                        for q in range(4):
                            for kk in range(4):
                                dst = gxh[q // 2][
                                    :, (q % 2) * 512:(q % 2) * 512 + 512]
                                nc.tensor.matmul(
                                    dst, hT[:, kk, :],
                                    whh_r[:, kk, q * 512:(q + 1) * 512],
                                    start=False, stop=(kk == 3),
                                    skip_group_check=True)
                        for kk in range(4):
                            nc.tensor.matmul(
                                emt[:, 64 * half:64 * half + K],
                                hT[:, kk, :], woutT_r[:, kk, :],
                                start=(kk == 0), stop=(kk == 3),
                                skip_group_check=True)

                        cst_new = lp.tile([B, H], F32, tag="cst", bufs=2,
                                          name=f"cst{s + 1}")
                        hT_new = lp.tile([128, 4, 128], F32R, tag="hT",
                                         bufs=2, name=f"hT{s + 1}")
                        for hf in range(2):
                            gsrc = gxh[hf]
                            hs = slice(256 * hf, 256 * hf + 256)
                            sio = lp.tile([B, 768], F32, tag=f"sio{hf}",
                                          bufs=2, name=f"sio{s}_{hf}")
                            nc.scalar.activation(sio[:],
                                                 gsrc[ro:ro + 64, 0:768],
                                                 AF.Sigmoid)
                            tg = lp.tile([B, 256], F32, tag=f"tg{hf}",
                                         bufs=2, name=f"tg{s}_{hf}")
                            nc.scalar.activation(tg[:],
                                                 gsrc[ro:ro + 64, 768:1024],
                                                 AF.Tanh)
                            ig = lp.tile([B, 256], F32, tag=f"ig{hf}",
                                         bufs=2, name=f"ig{s}_{hf}")
                            nc.vector.tensor_mul(ig[:], sio[:, 0:256], tg[:])
                            fc = lp.tile([B, 256], F32, tag=f"fc{hf}",
                                         bufs=2, name=f"fc{s}_{hf}")
                            nc.vector.tensor_mul(fc[:], sio[:, 256:512],
                                                  cst[:, hs])
                            nc.vector.tensor_add(cst_new[:, hs], ig[:],
                                                 fc[:])
                            tcc = lp.tile([B, 256], F32, tag=f"tcc{hf}",
                                          bufs=2, name=f"tcc{s}_{hf}")
                            nc.scalar.activation(tcc[:], cst_new[:, hs],
                                                 AF.Tanh)
                            hh = lp.tile([B, 256], F32, tag=f"hh{hf}",
                                         bufs=2, name=f"hh{s}_{hf}")
                            nc.vector.tensor_mul(hh[:], sio[:, 512:768],
                                                 tcc[:])
                            for c2 in range(2):
                                nc.tensor.transpose(
                                    tr_ps[:, (2 * hf + c2) * 64:
                                          (2 * hf + c2 + 1) * 64],
                                    hh[:, c2 * 128:(c2 + 1) * 128],
                                    ident[0:64, 0:64])
                            dst_hf = bass.AP(
                                tensor=hT_new[:].tensor,
                                offset=hT_new[:].offset + nhalf * 64
                                + 2 * hf * 128,
                                ap=[hT_new[:].ap[0], [128, 2], [1, 64]])
                            nc.vector.tensor_copy(
                                dst_hf,
                                tr_ps[:, 2 * hf * 64:
                                      (2 * hf + 2) * 64].rearrange(
                                    "p (a b) -> p a b", a=2))
                        cst = cst_new
                        hT = hT_new

                    em_sb = lp.tile([128, K], F32, tag="emsb", bufs=2,
                                    name=f"emsb{kt}")
                    nc.vector.tensor_tensor(em_sb[0:64, :],
                                            emt[0:64, 0:K],
                                            boutr_sb[0:64, :], op=OP.add)
                    nc.vector.tensor_tensor(em_sb[64:128, :],
                                            emt[64:128, 64:64 + K],
                                            boutr_sb[64:128, :], op=OP.add)
                    nc.gpsimd.indirect_dma_start(
                        out=em_view, out_offset=bass.IndirectOffsetOnAxis(
                            ap=emstidx_sb[:, kt:kt + 1], axis=0),
                        in_=em_sb[:], in_offset=None)

                # final emission for step T-1 (h from last half, m-half 0)
                for kk in range(4):
                    nc.tensor.matmul(emt[:, 0:K], hT[:, kk, :],
                                     woutT_r[:, kk, :], start=(kk == 0),
                                     stop=(kk == 3), skip_group_check=True)
                em_f = lp.tile([128, K], F32, tag="emsb", bufs=2,
                               name="emsbF")
                nc.vector.tensor_tensor(em_f[:], emt[:, 0:K],
                                        boutr_sb[:], op=OP.add)
                nc.gpsimd.indirect_dma_start(
                    out=em_view, out_offset=bass.IndirectOffsetOnAxis(
                        ap=emstidx_sb[:, NTILE:NTILE + 1], axis=0),
                    in_=em_f[:], in_offset=None)

            nc.gpsimd.collective_compute(
                "AllReduce", OP.add, replica_groups=g_all,
                ins=[em_loc.ap().opt()], outs=[em_shared.ap().opt()])

            # ---------------- max-plus scans (all cores) ----------------
            em6_view = bass.AP(tensor=em_shared.ap().tensor, offset=0,
                               ap=[[6, EMR * 8], [1, 6]])
            with tc.tile_pool(name="scan", bufs=2) as sp, \
                 tc.tile_pool(name="psc", bufs=2, space="PSUM") as psc:
                embk = {}

                def gather_step(k):
                    t_ = sp.tile([128, 6], F32, tag="embk", bufs=8,
                                 name=f"embk{k}")
                    nc.gpsimd.indirect_dma_start(
                        out=t_[:], out_offset=None, in_=em6_view,
                        in_offset=bass.IndirectOffsetOnAxis(
                            ap=scidx_sb[:, k:k + 1], axis=0))
                    embk[k] = t_

                def replicate(s6, k):
                    # block-diagonal rhs: bd[p=(jg,b), jg'*6+jl] =
                    # s6[p, jl] * (jg==jg'), then one MM against the
                    # constant b-replication stationary:
                    # srep[(jg',b), j] = sum_c patb[c,(jg',b)]*bd[c, j]
                    #                  = s6[(jg(j), b), jl(j)]
                    bd = sp.tile([128, K], F32, tag="bd", bufs=2,
                                 name=f"bd{k}")
                    s6b = bass.AP(tensor=s6[:].tensor, offset=s6[:].offset,
                                  ap=[s6[:].ap[0], [0, 8], [1, 6]])
                    nc.vector.tensor_tensor(bd[:], s6b, bmask_sb[:],
                                            op=OP.mult)
                    rep = psc.tile([128, K], F32, tag="srep",
                                   name=f"srep{k}")
                    nc.tensor.matmul(rep[:], patb_sb[:], bd[:],
                                     start=True, stop=True,
                                     skip_group_check=True)
                    return rep

                for kk_ in range(6):
                    gather_step(kk_)
                s6 = sp.tile([128, 6], F32, tag="s6", bufs=2, name="s6_0")
                nc.vector.tensor_tensor(s6[:], sinit6_sb[:],
                                        embk[0][:], op=OP.add)
                nc.scalar.dma_start(
                    bass.AP(tensor=score_loc.ap().tensor, offset=0,
                            ap=[[6, 8], [K, 16], [1, 6]]), s6[:])
                srep = replicate(s6, 0)

                for k in range(1, T):
                    if k + 5 < T:
                        gather_step(k + 5)
                    cand = sp.tile([128, 6, K], F32, tag="cand", bufs=2,
                                   name=f"cand{k}")
                    sr_b = bass.AP(tensor=srep[:].tensor,
                                   offset=srep[:].offset,
                                   ap=[srep[:].ap[0], [0, 6], [1, K]])
                    nc.vector.tensor_tensor(cand[:], transl_sb[:], sr_b,
                                            op=OP.add)
                    red = sp.tile([128, 6], F32, tag="red", bufs=2,
                                  name=f"red{k}")
                    nc.vector.tensor_reduce(red[:], cand[:],
                                            axis=mybir.AxisListType.X,
                                            op=OP.max)
                    s6 = sp.tile([128, 6], F32, tag="s6", bufs=2,
                                 name=f"s6_{k}")
                    nc.vector.tensor_tensor(s6[:], red[:],
                                            embk.pop(k)[:], op=OP.add)
                    nc.scalar.dma_start(
                        bass.AP(tensor=score_loc.ap().tensor,
                                offset=k * 16 * K,
                                ap=[[6, 8], [K, 16], [1, 6]]), s6[:])
                    if k < T - 1:
                        srep = replicate(s6, k)

            nc.gpsimd.collective_compute(
                "AllGather", OP.bypass, replica_groups=g_all,
                ins=[score_loc.ap().opt()], outs=[score_gath.ap().opt()])

            # ---------------- tags (argmax alpha+gamma-em) ----------------
            with tc.tile_pool(name="fin", bufs=2) as fp:
                # bulk-copy this pair's alpha/gamma rank blocks + em slice
                # to local DRAM (per-core variance only in gather offsets)
                for r_ in range(2):
                    g_ = fp.tile([128, 3072], F32, tag="blk", bufs=2,
                                 name=f"blk{r_}")
                    nc.gpsimd.indirect_dma_start(
                        out=g_[:], out_offset=None,
                        in_=bass.AP(tensor=score_gath.ap().tensor, offset=0,
                                    ap=[[3072, N_CORES * T * 16 * K // 3072],
                                        [1, 3072]]),
                        in_offset=bass.IndirectOffsetOnAxis(
                            ap=pridx_sb[:, r_:r_ + 1], axis=0))
                    nc.sync.dma_start(
                        bass.AP(tensor=score_pair.ap().tensor,
                                offset=r_ * T * 16 * K,
                                ap=[[3072, 128], [1, 3072]]), g_[:])
                for tc_ in range(4):
                    ge_ = fp.tile([128, 768], F32, tag="eblk", bufs=2,
                                  name=f"eblk{tc_}")
                    nc.gpsimd.indirect_dma_start(
                        out=ge_[:], out_offset=None,
                        in_=bass.AP(tensor=em_shared.ap().tensor, offset=0,
                                    ap=[[768, T * B * K // 768], [1, 768]]),
                        in_offset=bass.IndirectOffsetOnAxis(
                            ap=empidx_sb[:, tc_:tc_ + 1], axis=0))
                    nc.sync.dma_start(
                        bass.AP(tensor=em_pair.ap().tensor,
                                offset=tc_ * 128 * 768,
                                ap=[[768, 128], [1, 768]]), ge_[:])

                tags_i = fp.tile([128, 64], I32, tag="tagsi", bufs=1,
                                 name="tagsi")
                for b2 in range(2):
                    al = fp.tile([128, 32, K], F32, tag="al", bufs=2,
                                 name=f"al{b2}")
                    ga = fp.tile([128, 32, K], F32, tag="ga", bufs=2,
                                 name=f"ga{b2}")
                    em2 = fp.tile([128, 32, K], F32, tag="em2", bufs=2,
                                  name=f"em2_{b2}")
                    for tg in range(8):
                        t0 = tg * 64 + b2 * 32
                        k_lo = 511 - t0 - 31
                        nc.sync.dma_start(
                            al[16 * tg:16 * tg + 16, :, :], bass.AP(
                                tensor=score_pair.ap().tensor,
                                offset=t0 * 16 * K,
                                ap=[[K, 16], [16 * K, 32], [1, K]]))
                        nc.scalar.dma_start(
                            ga[16 * tg:16 * tg + 16, :, :], bass.AP(
                                tensor=score_pair.ap().tensor,
                                offset=(T + k_lo) * 16 * K,
                                ap=[[K, 16], [16 * K, 32], [1, K]]))
                        nc.gpsimd.dma_start(
                            em2[16 * tg:16 * tg + 16, :, :], bass.AP(
                                tensor=em_pair.ap().tensor,
                                offset=t0 * 16 * K,
                                ap=[[K, 16], [16 * K, 32], [1, K]]))
                    tot = fp.tile([128, 32, K], F32, tag="tot", bufs=2,
                                  name=f"tot{b2}")
                    ga_rev = bass.AP(
                        tensor=ga[:].tensor, offset=ga[:].offset + 31 * K,
                        ap=[ga[:].ap[0], [-K, 32], [1, K]])
                    nc.vector.tensor_tensor(tot[:], al[:], ga_rev, op=OP.add)
                    nc.vector.tensor_tensor(tot[:], tot[:], em2[:],
                                            op=OP.subtract)
                    mx = fp.tile([128, 32], F32, tag="mx", bufs=2,
                                 name=f"mx{b2}")
                    nc.vector.tensor_reduce(mx[:], tot[:],
                                            axis=mybir.AxisListType.X,
                                            op=OP.max)
                    msk = fp.tile([128, 32, K], F32, tag="msk", bufs=2,
                                  name=f"msk{b2}")
                    nc.vector.tensor_tensor(
                        msk[:], tot[:],
                        bass.AP(tensor=mx[:].tensor, offset=mx[:].offset,
                                ap=[mx[:].ap[0], [1, 32], [0, K]]),
                        op=OP.is_equal)
                    nc.vector.tensor_tensor(
                        msk[:], msk[:],
                        bass.AP(tensor=jshift[:].tensor,
                                offset=jshift[:].offset,
                                ap=[jshift[:].ap[0], [0, 32], [1, K]]),
                        op=OP.mult)
                    jm = fp.tile([128, 32], F32, tag="jm", bufs=2,
                                 name=f"jm{b2}")
                    nc.vector.tensor_reduce(jm[:], msk[:],
                                            axis=mybir.AxisListType.X,
                                            op=OP.min)
                    nc.vector.tensor_scalar_add(
                        tags_i[:, b2 * 32:b2 * 32 + 32], jm[:], 1000.0)

                # tags_i [p=(tg,b), u=b2*32+tl] -> tags_loc[b, tg*64+u]
                nc.sync.dma_start(
                    bass.AP(tensor=tags_loc.ap().tensor, offset=0,
                            ap=[[64, 8], [T, 16], [1, 64]]), tags_i[:])

            nc.gpsimd.collective_compute(
                "AllGather", OP.bypass, replica_groups=g_all,
                ins=[tags_loc.ap().opt()], outs=[tags_gath.ap().opt()])

            with tc.tile_pool(name="out", bufs=1) as op_:
                tags_sb = op_.tile([B, T], I32)
                for p4 in range(4):
                    nc.sync.dma_start(
                        tags_sb[16 * p4:16 * p4 + 16, :],
                        bass.AP(tensor=tags_gath.ap().tensor,
                                offset=(2 * p4) * 16 * T,
                                ap=[[T, 16], [1, T]]))
                nc.sync.dma_start(tags_ap[:, :], tags_sb[:])

    nc.compile()
    return nc


def _host_prep(inputs):
    x = np.asarray(inputs["x"]).astype(np.int64)
    emb = np.asarray(inputs["emb"], np.float32)
    trans = np.asarray(inputs["crf_trans"], np.float32)

    maps = []
    p = np.arange(128)
    for core in range(N_CORES):
        m = {}
        is_lstm = core < 2
        d = "f" if core == 0 else "b"
        if is_lstm:
            x_eff = x if core == 0 else np.ascontiguousarray(x[:, ::-1])
            xe = emb[x_eff]                       # [B, T, E]
            xseq = xe.transpose(1, 0, 2).reshape(NTILE, 128, E)
            xeT = np.ascontiguousarray(
                xseq.transpose(0, 2, 1)).reshape(NTILE * 2 * 128, 128)
            m["xeT"] = xeT
            wih = np.asarray(inputs[f"w_ih_{d}"], np.float32)[_GPERM].T
            m["wih"] = np.ascontiguousarray(
                wih.reshape(2, 128, G).transpose(1, 0, 2)).reshape(128, 2 * G)
            whh = np.asarray(inputs[f"w_hh_{d}"], np.float32)[_GPERM].T
            m["whh"] = np.ascontiguousarray(
                whh.reshape(4, 128, G).transpose(1, 0, 2)).reshape(128, 4 * G)
            m["bias"] = np.asarray(
                inputs[f"b_{d}"], np.float32)[_GPERM].reshape(1, G)
            wo = np.asarray(inputs["w_out"], np.float32)
            half = wo[:, :H] if core == 0 else wo[:, H:]
            m["woutT"] = np.ascontiguousarray(
                half.T.reshape(4, 128, K).transpose(1, 0, 2)).reshape(
                128, 4 * K)
            m["boutr"] = (np.tile(np.asarray(inputs["b_out"], np.float32),
                                  (128, 1)) if core == 0
                          else np.zeros((128, K), np.float32))
        else:
            m["xeT"] = np.zeros((NTILE * 2 * 128, 128), np.float32)
            m["wih"] = np.zeros((128, 2 * G), np.float32)
            m["whh"] = np.zeros((128, 4 * G), np.float32)
            m["bias"] = np.zeros((1, G), np.float32)
            m["woutT"] = np.zeros((128, 4 * K), np.float32)
            m["boutr"] = np.zeros((128, K), np.float32)

        # em store indices: tile kt rows r: step = 2kt-1 + r//64 (fwd t=step,
        # bwd t = T-1-step), row = t*B + b, b = r % 64
        tmap = (np.arange(T) if core % 2 == 0 else T - 1 - np.arange(T))
        st = np.empty((128, NTILE + 1), np.int64)
        lstm_tmap = np.arange(T) if core != 1 else T - 1 - np.arange(T)
        trash = T * B + (p % 64)
        for kt in range(NTILE):
            step = 2 * kt - 1 + p // 64
            st[:, kt] = np.where(step < 0, trash,
                                 lstm_tmap[np.clip(step, 0, T - 1)] * B
                                 + (p % 64))
        st[:, NTILE] = np.where(p < 64, lstm_tmap[T - 1] * B + (p % 64),
                                trash)
        m["emstidx"] = st.astype(np.int32)

        # scan inputs: pair owns b in [16*pair, 16*pair+16)
        pair = core // 2
        is_alpha = (core % 2 == 0)
        ig = p // 16
        bl = p % 16
        bg = 16 * pair + bl
        tr = trans if is_alpha else np.ascontiguousarray(trans.T)
        # transl[p=(jg,b), il, i] = tr[i, j=jg*6+il]  (j-split layout)
        m["transl"] = np.ascontiguousarray(
            tr.T[(ig[:, None] * 6 + np.arange(6)[None, :])]).reshape(
            128, 6 * K)
        m["patb"] = (np.arange(128)[:, None] % 16
                     == np.arange(128)[None, :] % 16).astype(np.float32)
        m["bmask"] = (np.arange(128)[:, None] // 16
                      == np.arange(K)[None, :] // 6).astype(np.float32)
        sv = np.asarray(inputs["crf_start" if is_alpha else "crf_end"],
                        np.float32)
        s6 = np.empty((128, 6), np.float32)
        for il in range(6):
            s6[:, il] = sv[ig * 6 + il]
        m["sinit6"] = s6
        # scidx[p, k] = row of em viewed [T*B*8, 6]: t_order(k)*B*8 + bg*8+ig
        korder = np.arange(T) if is_alpha else T - 1 - np.arange(T)
        m["scidx"] = (korder[None, :] * (B * 8) + (bg * 8 + ig)[:, None]
                      ).astype(np.int32)
        # tags bulk-copy offsets: pridx (rank blocks, 3072-el units),
        # empidx (em t-rows for pair's b-slice, 768-el units)
        pr = np.empty((128, 2), np.int64)
        pr[:, 0] = 2 * pair * 128 + p
        pr[:, 1] = (2 * pair + 1) * 128 + p
        m["pridx"] = pr.astype(np.int32)
        emp = np.empty((128, 4), np.int64)
        for tc_ in range(4):
            emp[:, tc_] = 4 * (tc_ * 128 + p) + pair
        m["empidx"] = emp.astype(np.int32)
        maps.append(m)
    return maps


_NC_CACHE = {}


def _get_nc():
    if "nc" not in _NC_CACHE:
        _NC_CACHE["nc"] = _build_nc()
    return _NC_CACHE["nc"]


def kernel(**inputs):
    nc = _get_nc()
    maps = _host_prep(inputs)
    res = run_bass_kernel_spmd(nc, maps, core_ids=list(range(N_CORES)))
    return res.results[0]["tags"].astype(np.int32)
